# revision 1
# baseline (speedup 1.0000x reference)
"""Self-contained Trainium2 Bass kernel for the ChemGPT problem.

Data-parallel over batch: each of the 8 NeuronCores runs the full 8-layer
transformer on one batch element. Embedding gather happens on host (tiny);
everything else (LayerNorm, QKV, causal attention, MLP, final LN, logits)
runs on-device in feature-major layout with fp32r matmuls.
"""
import numpy as np
import bass_rust

"""Bass/Tile kernel builder for a GPT block stack (feature-major activations).

Per-core program: x0T [C,T] -> L transformer blocks -> final LN -> logits [T,V].
All matmuls run as fp32r (full PE rate at N>=256). Activations feeding matmuls
are produced directly in float32r by their producer (ACT/DVE rounding).
LayerNorm stats use GPSIMD partition_all_reduce (cross-partition sum, output
broadcast to all partitions). Causal softmax is computed in "scoresT" layout
[k, q] so no transposes are needed anywhere; the per-query normalizer 1/s is
broadcast across partitions with a K=1 ones matmul.
"""

from contextlib import ExitStack

import concourse.bass as bass
import concourse.bass_isa as bass_isa
import concourse.mybir as mybir
import concourse.tile as tile

F32 = mybir.dt.float32
F32R = mybir.dt.float32r
AF = mybir.ActivationFunctionType
OP = mybir.AluOpType


class Cfg:
    def __init__(self, T=511, C=1024, H=16, L=8, FF=4096, V=128):
        self.T, self.C, self.H, self.L, self.FF, self.V = T, C, H, L, FF, V
        self.TP = T + (T % 2)  # padded (even) streaming width for fp32r
        self.D = C // H
        assert self.D == 64, "head-pair packing assumes D=64"
        self.CT = C // 128
        self.FT = FF // 128
        self.TT = (T + 127) // 128
        self.NCH = max(1, FF // 1024)  # W1/W2 are processed in 1024-wide chunks
        self.FPC = self.FT // self.NCH  # f-tiles per chunk (8)
        # attention tile geometry: per key-tile kt, the streamed q-window
        # [q0, T) must be >=256 wide for fp32r full rate
        self.kt_geo = []
        for kt in range(self.TT):
            k0 = 128 * kt
            ksize = min(128, T - k0)
            q0 = k0 if (self.TP - k0) >= 256 else max(0, self.TP - 256)
            self.kt_geo.append((k0, ksize, q0, self.TP - q0))
        # mask windows (relative to q0): cover q in [q0, k0+ksize)
        self.mask_off = []
        off = 0
        for (k0, ksize, q0, _n) in self.kt_geo:
            w = k0 + ksize - q0
            self.mask_off.append((off, w))
            off += w
        self.mask_w = off


PAIRW = 193  # per head-pair stationary region: [v_even(64) | ones | ones | zeros(63) | v_odd(64)]


def host_vinit(cfg):
    """Constant init pattern for the packed V stationary buffer.

    Layout per pair p (width 193):
      cols [0:64)    v of even head (runtime)
      col  64        ones  (sum row for even head -> psum row 64)
      col  65        ones  (sum row for odd head -> psum row 0)
      cols [66:129)  zeros (psum rows 1..63 for odd head)
      cols [129:193) v of odd head (runtime -> psum rows 64..127)
    """
    import numpy as np
    NP = cfg.H // 2
    v = np.zeros((128, NP * PAIRW), dtype=np.float32)
    for p in range(NP):
        v[:, p * PAIRW + 64] = 1.0
        v[:, p * PAIRW + 65] = 1.0
    return v


def host_masks(cfg):
    import numpy as np
    m = np.zeros((128, cfg.mask_w), dtype=np.float32)
    for kt, (k0, ksize, q0, _n) in enumerate(cfg.kt_geo):
        off, w = cfg.mask_off[kt]
        for kl in range(ksize):
            for j in range(w):
                q = q0 + j
                if q >= k0 + kl:
                    m[kl, off + j] = 1.0
    return m


def build_gpt(cfg, dual_psum_tt=False):
    T, C, H, L, FF, V = cfg.T, cfg.C, cfg.H, cfg.L, cfg.FF, cfg.V
    TP = cfg.TP
    CT, FT, TT, D = cfg.CT, cfg.FT, cfg.TT, cfg.D
    HT = H // 2  # head-pairs per feature tile == CT

    nc = bass.Bass(target_bir_lowering=False)

    # ---- DRAM I/O ----
    x0T_d = nc.dram_tensor("x0T", [C, T], F32, kind="ExternalInput")
    Wq_d = nc.dram_tensor("Wq", [L, C, C], F32R, kind="ExternalInput")
    Wk_d = nc.dram_tensor("Wk", [L, C, C], F32R, kind="ExternalInput")
    Wv_d = nc.dram_tensor("Wv", [L, C, C], F32R, kind="ExternalInput")
    Wp_d = nc.dram_tensor("Wp", [L, C, C], F32R, kind="ExternalInput")
    W1_d = nc.dram_tensor("W1", [L, C, FF], F32R, kind="ExternalInput")
    W2_d = nc.dram_tensor("W2", [L, FF, C], F32R, kind="ExternalInput")
    bq_d = nc.dram_tensor("bq", [L, C], F32, kind="ExternalInput")
    bk_d = nc.dram_tensor("bk", [L, C], F32, kind="ExternalInput")
    bv_d = nc.dram_tensor("bv", [L, C], F32R, kind="ExternalInput")
    bp_d = nc.dram_tensor("bp", [L, C], F32, kind="ExternalInput")
    b1_d = nc.dram_tensor("b1", [L, FF], F32, kind="ExternalInput")
    b2_d = nc.dram_tensor("b2", [L, C], F32, kind="ExternalInput")
    ln1w_d = nc.dram_tensor("ln1w", [L, C], F32, kind="ExternalInput")
    ln1b_d = nc.dram_tensor("ln1b", [L, C], F32, kind="ExternalInput")
    ln2w_d = nc.dram_tensor("ln2w", [L, C], F32, kind="ExternalInput")
    ln2b_d = nc.dram_tensor("ln2b", [L, C], F32, kind="ExternalInput")
    lnfw_d = nc.dram_tensor("lnfw", [C], F32, kind="ExternalInput")
    lnfb_d = nc.dram_tensor("lnfb", [C], F32, kind="ExternalInput")
    headT_d = nc.dram_tensor("headT", [C, V], F32R, kind="ExternalInput")
    mask_d = nc.dram_tensor("maskcat", [128, cfg.mask_w], F32R, kind="ExternalInput")
    ones_d = nc.dram_tensor("onesmat", [128, 128], F32R, kind="ExternalInput")
    vinit_d = nc.dram_tensor("vinit", [128, (H // 2) * PAIRW], F32R,
                             kind="ExternalInput")
    out_d = nc.dram_tensor("out", [T, V], F32, kind="ExternalOutput")

    ctx = ExitStack()
    with ctx:
        ctx.enter_context(nc.allow_low_precision(
            reason="float32r tiles feed fp32r matmuls; 4-byte near-fp32"))
        tc = ctx.enter_context(tile.TileContext(nc))
        px = ctx.enter_context(tc.tile_pool(name="px", bufs=2))
        ph = ctx.enter_context(tc.tile_pool(name="ph", bufs=1))
        pq = ctx.enter_context(tc.tile_pool(name="pq", bufs=1))
        pk = ctx.enter_context(tc.tile_pool(name="pk", bufs=1))
        py = ctx.enter_context(tc.tile_pool(name="py", bufs=1))
        pv = ctx.enter_context(tc.tile_pool(name="pv", bufs=1))
        pe = ctx.enter_context(tc.tile_pool(name="pe", bufs=2))
        p1 = ctx.enter_context(tc.tile_pool(name="p1", bufs=1))
        pw = ctx.enter_context(tc.tile_pool(name="pw", bufs=3))
        pst = ctx.enter_context(tc.tile_pool(name="pst", bufs=1))
        psm = ctx.enter_context(tc.tile_pool(name="psm", bufs=1))
        pbi = ctx.enter_context(tc.tile_pool(name="pbi", bufs=2))
        pps = ctx.enter_context(tc.tile_pool(name="pps", bufs=8, space="PSUM"))

        def ps_tile(name):
            t = pps.tile([128, 512], F32, name=name, tag="ps")
            return t

        # ---- constants ----
        mask_sb = psm.tile([128, cfg.mask_w], F32R, name="mask_sb")
        nc.sync.dma_start(mask_sb[:], mask_d[:])
        ones_sb = psm.tile([128, 128], F32R, name="ones_sb")
        nc.sync.dma_start(ones_sb[:], ones_d[:])
        head_sb = psm.tile([128, CT, V], F32R, name="head_sb")
        for c in range(CT):
            nc.sync.dma_start(head_sb[:, c, :], headT_d[128 * c:128 * c + 128, :])
        lnfw_t = psm.tile([128, CT], F32, name="lnfw_t")
        nc.sync.dma_start(lnfw_t[:], lnfw_d.rearrange("(o p) -> p o", p=128))
        lnfb_t = psm.tile([128, CT], F32, name="lnfb_t")
        nc.sync.dma_start(lnfb_t[:], lnfb_d.rearrange("(o p) -> p o", p=128))

        # ---- load x0 ----
        x_cur = px.tile([128, CT, TP], F32, name="x_in", tag="x")
        for c in range(CT):
            nc.sync.dma_start(x_cur[:, c, 0:T], x0T_d[128 * c:128 * c + 128, :])
            if TP > T:
                nc.vector.memset(x_cur[:, c, T:TP], 0.0)

        v_aug = pv.tile([128, TT, (H // 2) * PAIRW], F32R, name="vA", tag="vA")
        for kt in range(TT):
            nc.sync.dma_start(v_aug[:, kt, :], vinit_d[:])

        def layer_bias(name, dram, l, width):
            t = pbi.tile([128, width], F32, name=f"{name}_{l}", tag=name)
            nc.sync.dma_start(t[:], dram[l].rearrange("(o p) -> p o", p=128))
            return t

        def layernorm(xt, w_t, b_t, tagpfx):
            """xt [128, CT, T] f32 -> h [128, CT, T] f32r.

            Cross-partition sums via ones-column matmuls (PE); per-token
            stats computed on single-partition rows; broadcast back across
            partitions via K=1 ones-row matmuls into PSUM."""
            h = ph.tile([128, CT, TP], F32R, name=f"h_{tagpfx}", tag="h")
            acc = pst.tile([128, TP], F32R, name=f"acc_{tagpfx}", tag="acc")
            acc2 = pst.tile([128, TP], F32R, name=f"acc2_{tagpfx}", tag="acc2")
            if CT > 1:
                nc.vector.tensor_add(acc[:], xt[:, 0, :], xt[:, 1, :])
            else:
                nc.vector.tensor_copy(acc[:], xt[:, 0, :])
            for c in range(2, CT):
                nc.vector.tensor_add(acc[:], acc[:], xt[:, c, :])
            nc.scalar.activation(acc2[:], xt[:, 0, :], AF.Square)
            for c in range(1, CT):
                sqt = pst.tile([128, TP], F32, name=f"sqt_{tagpfx}_{c}", tag="sqt",
                               bufs=2)
                nc.scalar.activation(sqt[:], xt[:, c, :], AF.Square)
                nc.vector.tensor_add(acc2[:], acc2[:], sqt[:])
            ps_s1 = ps_tile(f"ps_s1_{tagpfx}")
            nc.tensor.matmul(ps_s1[0:1, :TP], ones_sb[:, 0:1], acc[:],
                             start=True, stop=True)
            ps_s2 = ps_tile(f"ps_s2_{tagpfx}")
            nc.tensor.matmul(ps_s2[0:1, :TP], ones_sb[:, 0:1], acc2[:],
                             start=True, stop=True)
            m_row = pst.tile([1, TP], F32R, name=f"mrow_{tagpfx}", tag="mrow")
            nc.scalar.activation(m_row[0:1, :], ps_s1[0:1, :TP], AF.Copy,
                                 scale=1.0 / C)
            t1r = pst.tile([1, TP], F32, name=f"t1r_{tagpfx}", tag="t1r")
            nc.vector.tensor_mul(t1r[0:1, :], m_row[0:1, :], m_row[0:1, :])
            nc.vector.scalar_tensor_tensor(t1r[0:1, :], ps_s2[0:1, :TP], 1.0 / C,
                                           t1r[0:1, :], OP.mult, OP.subtract)
            nc.vector.tensor_scalar_add(t1r[0:1, :], t1r[0:1, :], 1e-5)
            t2r = pst.tile([1, TP], F32, name=f"t2r_{tagpfx}", tag="t2r")
            nc.scalar.activation(t2r[0:1, :], t1r[0:1, :], AF.Sqrt)
            rstd_row = pst.tile([1, TP], F32R, name=f"rrow_{tagpfx}", tag="rrow")
            nc.vector.reciprocal(rstd_row[0:1, :], t2r[0:1, :])
            ps_mbc = ps_tile(f"ps_mbc_{tagpfx}")
            nc.tensor.matmul(ps_mbc[:, :TP], ones_sb[0:1, :], m_row[0:1, :],
                             start=True, stop=True)
            ps_rbc = ps_tile(f"ps_rbc_{tagpfx}")
            nc.tensor.matmul(ps_rbc[:, :TP], ones_sb[0:1, :], rstd_row[0:1, :],
                             start=True, stop=True)
            for c in range(CT):
                t1 = pst.tile([128, TP], F32, name=f"lnt1_{tagpfx}_{c}", tag="lnt1",
                              bufs=2)
                nc.vector.tensor_sub(t1[:], xt[:, c, :], ps_mbc[:, :TP])
                nc.vector.tensor_mul(t1[:], t1[:], ps_rbc[:, :TP])
                nc.scalar.activation(h[:, c, :], t1[:], AF.Identity,
                                     bias=b_t[:, c:c + 1], scale=w_t[:, c:c + 1])
            return h

        qk_scale = 1.0 / (D ** 0.5)

        for l in range(L):
            ln1w_t = layer_bias("ln1w", ln1w_d, l, CT)
            ln1b_t = layer_bias("ln1b", ln1b_d, l, CT)
            h = layernorm(x_cur, ln1w_t, ln1b_t, f"l{l}a")

            bq_t0 = layer_bias("bq", bq_d, l, CT)
            bq_t = pbi.tile([128, CT], F32, name=f"bqs_{l}", tag="bqs")
            nc.vector.tensor_scalar_mul(bq_t[:], bq_t0[:], qk_scale)
            bk_t = layer_bias("bk", bk_d, l, CT)
            bp_t = layer_bias("bp", bp_d, l, CT)
            bv_row = pbi.tile([1, C], F32R, name=f"bvr_{l}", tag="bvr")
            nc.sync.dma_start(bv_row[:], bv_d[l][None, :])

            # ---- Q projection (feature-major out) ----
            qT = pq.tile([128, CT, TP], F32R, name=f"qT_{l}", tag="qT")
            kT = pk.tile([128, CT, TP], F32R, name=f"kT_{l}", tag="kT")
            for (WT, dst, bias_t, scale) in ((Wq_d, qT, bq_t, qk_scale),
                                             (Wk_d, kT, bk_t, 1.0)):
                pss = [ps_tile(f"psq{o}_{l}") for o in range(CT)]
                for c in range(CT):
                    wt = pw.tile([128, C], F32R, name=f"w_{l}_{c}", tag="w")
                    nc.sync.dma_start(wt[:], WT[l, 128 * c:128 * c + 128, :])
                    for o in range(CT):
                        nc.tensor.matmul(pss[o][:, :TP], wt[:, 128 * o:128 * o + 128],
                                         h[:, c, :], start=(c == 0),
                                         stop=(c == CT - 1))
                for o in range(CT):
                    nc.scalar.activation(dst[:, o, :], pss[o][:, :TP], AF.Identity,
                                         bias=bias_t[:, o:o + 1], scale=scale)

            # ---- V projection (token-major out) ----
            VW = min(512, C)
            NJ = C // VW
            psv = [ps_tile(f"psv{t}_{j}_{l}") for t in range(TT) for j in range(NJ)]
            for c in range(CT):
                wt = pw.tile([128, C], F32R, name=f"wv_{l}_{c}", tag="w")
                nc.sync.dma_start(wt[:], Wv_d[l, 128 * c:128 * c + 128, :])
                for t in range(TT):
                    tsz = min(128, T - 128 * t)
                    for j in range(NJ):
                        nc.tensor.matmul(
                            psv[t * NJ + j][:tsz, :VW],
                            h[:, c, 128 * t:128 * t + tsz],
                            wt[:, VW * j:VW * j + VW],
                            start=(c == 0), stop=False)
            for t in range(TT):
                tsz = min(128, T - 128 * t)
                for j in range(NJ):
                    nc.tensor.matmul(psv[t * NJ + j][:tsz, :VW],
                                     ones_sb[0:1, 0:tsz],
                                     bv_row[0:1, VW * j:VW * j + VW],
                                     start=False, stop=True)
                    for hh in range(VW * j // 64, VW * (j + 1) // 64):
                        p, odd = hh // 2, hh % 2
                        dst0 = p * PAIRW + (129 if odd else 0)
                        nc.vector.tensor_copy(
                            v_aug[:tsz, t, dst0:dst0 + 64],
                            psv[t * NJ + j][:tsz, 64 * hh - VW * j:
                                            64 * hh - VW * j + 64])

            # ---- attention (scoresT layout, per head) ----
            yT = py.tile([128, CT, TP], F32R, name=f"yT_{l}", tag="yT")
            for hh in range(H):
                pbase = 64 * (hh % 2)
                ft = hh // 2
                exps = []
                for kt in range(TT):
                    k0, ksize, q0, N = cfg.kt_geo[kt]
                    ps_s = ps_tile(f"pss{hh}_{kt}_{l}")
                    nc.tensor.matmul(
                        ps_s[:ksize, :N],
                        kT[pbase:pbase + 64, ft, k0:k0 + ksize],
                        qT[pbase:pbase + 64, ft, q0:TP],
                        start=True, stop=True)
                    ex = pe.tile([128, N], F32R, name=f"ex{hh}_{kt}_{l}",
                                 tag=f"ex{kt}")
                    nc.scalar.activation(ex[:ksize, :], ps_s[:ksize, :N], AF.Exp)
                    moff, mw = cfg.mask_off[kt]
                    nc.vector.tensor_mul(ex[:ksize, 0:mw], ex[:ksize, 0:mw],
                                         mask_sb[:ksize, moff:moff + mw])
                    exps.append(ex)
                # attV: packed stationary yields y rows at the head's final
                # partitions plus the per-q normalizer row, in one matmul.
                ps_y = ps_tile(f"psy{hh}_{l}")
                p, odd = hh // 2, hh % 2
                srow = 0 if odd else 64
                for kt in range(TT):
                    k0, ksize, q0, N = cfg.kt_geo[kt]
                    if odd:
                        lhsT = v_aug[:ksize, kt, p * PAIRW + 65:p * PAIRW + PAIRW]
                        outw = 128
                    else:
                        lhsT = v_aug[:ksize, kt, p * PAIRW:p * PAIRW + 65]
                        outw = 65
                    nc.tensor.matmul(ps_y[0:outw, q0:TP], lhsT,
                                     exps[kt][:ksize, :],
                                     start=(kt == 0), stop=(kt == TT - 1))
                rs = pst.tile([128, TP], F32R, name=f"rs_{hh}_{l}", tag="rs", bufs=2)
                nc.vector.reciprocal(rs[srow:srow + 1, :], ps_y[srow:srow + 1, 0:TP])
                ps_bc = ps_tile(f"psbc{hh}_{l}")
                nc.tensor.matmul(ps_bc[:, :TP], ones_sb[srow:srow + 1, :],
                                 rs[srow:srow + 1, :], start=True, stop=True)
                rsb = pst.tile([128, TP], F32, name=f"rsb_{hh}_{l}", tag="rsb",
                               bufs=2)
                nc.scalar.activation(rsb[:], ps_bc[:, :TP], AF.Copy)
                nc.vector.tensor_mul(yT[pbase:pbase + 64, ft, :],
                                     ps_y[pbase:pbase + 64, 0:TP],
                                     rsb[pbase:pbase + 64, :])

            # ---- output projection + residual ----
            x_mid = px.tile([128, CT, TP], F32, name=f"xm_{l}", tag="x")
            psp = [ps_tile(f"psp{o}_{l}") for o in range(CT)]
            for c in range(CT):
                wt = pw.tile([128, C], F32R, name=f"wp_{l}_{c}", tag="w")
                nc.sync.dma_start(wt[:], Wp_d[l, 128 * c:128 * c + 128, :])
                for o in range(CT):
                    nc.tensor.matmul(psp[o][:, :TP], wt[:, 128 * o:128 * o + 128],
                                     yT[:, c, :], start=(c == 0), stop=(c == CT - 1))
            for o in range(CT):
                nc.vector.scalar_tensor_tensor(x_mid[:, o, :], psp[o][:, :TP],
                                               bp_t[:, o:o + 1], x_cur[:, o, :],
                                               OP.add, OP.add)

            # ---- MLP ----
            ln2w_t = layer_bias("ln2w", ln2w_d, l, CT)
            ln2b_t = layer_bias("ln2b", ln2b_d, l, CT)
            h2 = layernorm(x_mid, ln2w_t, ln2b_t, f"l{l}b")
            b1_t = pbi.tile([128, FT], F32, name=f"b1_{l}", tag="b1")
            nc.sync.dma_start(b1_t[:], b1_d[l].rearrange("(o p) -> p o", p=128))
            b2_t = layer_bias("b2", b2_d, l, CT)

            x_new = px.tile([128, CT, TP], F32, name=f"xn_{l}", tag="x")
            for o in range(CT):
                nc.scalar.activation(x_new[:, o, :], x_mid[:, o, :], AF.Identity,
                                     bias=b2_t[:, o:o + 1])
            for j in range(cfg.NCH):
                h1c = p1.tile([128, cfg.FPC, TP], F32R, name=f"h1_{l}_{j}", tag="h1")
                ps1 = [ps_tile(f"ps1{f}_{l}_{j}") for f in range(cfg.FPC)]
                for c in range(CT):
                    wt = pw.tile([128, 128 * cfg.FPC], F32R, name=f"w1_{l}_{j}_{c}",
                                 tag="w")
                    nc.sync.dma_start(
                        wt[:], W1_d[l, 128 * c:128 * c + 128,
                                    1024 * j:1024 * j + 128 * cfg.FPC])
                    for f in range(cfg.FPC):
                        nc.tensor.matmul(ps1[f][:, :TP], wt[:, 128 * f:128 * f + 128],
                                         h2[:, c, :], start=(c == 0),
                                         stop=(c == CT - 1))
                for f in range(cfg.FPC):
                    nc.scalar.activation(h1c[:, f, :], ps1[f][:, :TP], AF.Gelu,
                                         bias=b1_t[:, j * cfg.FPC + f:
                                                   j * cfg.FPC + f + 1])
                ps2 = [ps_tile(f"ps2{o}_{l}_{j}") for o in range(CT)]
                for f in range(cfg.FPC):
                    ftg = j * cfg.FPC + f
                    wt = pw.tile([128, C], F32R, name=f"w2_{l}_{j}_{f}", tag="w")
                    nc.sync.dma_start(wt[:], W2_d[l, 128 * ftg:128 * ftg + 128, :])
                    for o in range(CT):
                        nc.tensor.matmul(ps2[o][:, :TP], wt[:, 128 * o:128 * o + 128],
                                         h1c[:, f, :], start=(f == 0),
                                         stop=(f == cfg.FPC - 1))
                for o in range(CT):
                    nc.vector.tensor_add(x_new[:, o, :], x_new[:, o, :],
                                         ps2[o][:, :TP])
            x_cur = x_new

        # ---- final LN + logits ----
        hf = layernorm(x_cur, lnfw_t, lnfb_t, "lf")
        for t in range(TT):
            tsz = min(128, T - 128 * t)
            ps_lg = ps_tile(f"pslg{t}")
            for c in range(CT):
                nc.tensor.matmul(ps_lg[:tsz, :V], hf[:, c, 128 * t:128 * t + tsz],
                                 head_sb[:, c, :], start=(c == 0),
                                 stop=(c == CT - 1))
            lg = psm.tile([128, V], F32, name=f"lg_{t}", tag="lg", bufs=2)
            nc.scalar.activation(lg[:tsz, :], ps_lg[:tsz, :V], AF.Copy)
            nc.sync.dma_start(out_d[128 * t:128 * t + tsz, :], lg[:tsz, :])

    return nc



def split_multiwaits(nc):
    """Walrus enforces 1 sync-wait per instruction (2 for EventSemaphore).
    Tile's tail drain (and occasionally other insts) can carry more; split the
    excess into preceding same-engine NOPs, each carrying one wait."""
    n_fixed = 0
    for f in nc.m.functions:
        for bb in f.blocks:
            insts = list(bb.instructions)
            newlist = []
            changed = False
            for inst in insts:
                si = inst.sync_info
                cap = 2 if isinstance(inst, mybir.InstEventSemaphore) else 1
                if si is not None and si.on_wait is not None and len(si.on_wait) > cap:
                    waits = list(si.on_wait)
                    eng = inst.engine
                    extra, keep = waits[:-cap], waits[-cap:]
                    for k, w in enumerate(extra):
                        nop = mybir.InstNoOp(name=f"wsplit_{inst.name}_{k}", ins=[], outs=[])
                        nop.engine = eng
                        nop.sync_info = bass_rust.SyncInfo(on_wait=[w], on_update=[])
                        newlist.append(nop)
                    inst.sync_info = bass_rust.SyncInfo(on_wait=keep, on_update=list(si.on_update or []))
                    n_fixed += 1
                    changed = True
                newlist.append(inst)
            if changed:
                bb.instructions = newlist
    return n_fixed


_B, _T, _C, _H, _L, _V = 8, 511, 1024, 16, 8, 128
_FF = 4 * _C


def _prep_shared(inputs):
    cfg = Cfg(T=_T, C=_C, H=_H, L=_L, FF=_FF, V=_V)
    f32 = lambda a: np.ascontiguousarray(np.asarray(a), dtype=np.float32)
    shared = {
        "Wq": f32(inputs["Wq"]), "Wk": f32(inputs["Wk"]),
        "Wv": f32(inputs["Wv"]), "Wp": f32(inputs["Wp"]),
        "W1": f32(inputs["W1"]), "W2": f32(inputs["W2"]),
        "bq": f32(inputs["bq"]), "bk": f32(inputs["bk"]),
        "bv": f32(inputs["bv"]), "bp": f32(inputs["bp"]),
        "b1": f32(inputs["b1"]), "b2": f32(inputs["b2"]),
        "ln1w": f32(inputs["ln1_w"]), "ln1b": f32(inputs["ln1_b"]),
        "ln2w": f32(inputs["ln2_w"]), "ln2b": f32(inputs["ln2_b"]),
        "lnfw": f32(inputs["lnf_w"]), "lnfb": f32(inputs["lnf_b"]),
        "headT": np.ascontiguousarray(f32(inputs["head_w"]).T),
        "maskcat": host_masks(cfg),
        "onesmat": np.ones((128, 128), dtype=np.float32),
        "vinit": host_vinit(cfg),
    }
    return cfg, shared


def kernel(**inputs):
    cfg, shared = _prep_shared(inputs)
    idx = np.asarray(inputs["idx"]).astype(np.int64)
    tok = np.asarray(inputs["tok_emb"], dtype=np.float32)
    pos = np.asarray(inputs["pos_emb"], dtype=np.float32)
    x0 = tok[idx] + pos[None, :, :]              # [B, T, C] fp32
    in_maps = []
    for b in range(_B):
        m = dict(shared)
        m["x0T"] = np.ascontiguousarray(x0[b].T)  # [C, T]
        in_maps.append(m)

    nc = build_gpt(cfg)
    split_multiwaits(nc)
    from concourse.bass_utils import run_bass_kernel_spmd
    res = run_bass_kernel_spmd(nc, in_maps, core_ids=list(range(_B)))
    out = np.concatenate([res.results[b]["out"] for b in range(_B)], axis=0)
    return out.astype(np.float32)

